# revision 11
# baseline (speedup 1.0000x reference)
"""Trainium2 Bass kernel for nn_Controller (LSTM controller + categorical sampling).

Strategy
--------
Data-parallel over the batch: 4096 rows -> 8 NeuronCores x 512 rows; all
LSTM / linear / embedding parameters replicated per core.

Per core, per step (transposed layout [feature, batch]):
  gates^T[4H, B] = W_hh @ h^T + (emb_tab @ W_ih^T)^T-fused one-hot matmul + bias
  LSTM elementwise (tanh-form sigmoid on the ACT engine)
  logits[B, 100]  = h @ W_sel^T + b_sel
  probs           = softmax(logits)           (output)
  tok             = argmax(logits + Gumbel)   (== jax.random.categorical)

The Gumbel noise per step is a pure function of jax.random.PRNGKey(42) and
is precomputed on the host, so sampling is an argmax on the device.

All heavy matmuls use an exact fp16 hi/lo split (w = w0 + w1/2048,
h = h0 + h1/2048): three fp16 passes (w0h0 | w0h1 + w1h0) accumulated into
two PSUM banks, combined as main + cross/2048.  fp16 products are exact in
the fp32 PSUM accumulator, so this reaches full-fp32 accuracy at 1 cycle/row
instead of fp32's 4 cycles/row (measured max err 2.5e-7 vs 3.7e-7 for fp32).
"""

import sys
import time

sys.path.insert(0, "/opt/trn_rl_repo")

import numpy as np
from contextlib import ExitStack

import concourse.bacc as bacc
import concourse.tile as tile
from concourse import mybir
import concourse.bass as bass

FP32 = mybir.dt.float32
FP16 = mybir.dt.float16
AF = mybir.ActivationFunctionType
OP = mybir.AluOpType

B, E, H, NOPS, T = 4096, 512, 2048, 100, 20
NCORES = 8
BL = B // NCORES          # 512 rows per core
BT = BL // 128            # 4 batch tiles
KC = H // 128             # 16 contraction chunks
SC = 2048.0               # lo-part scale (2^11)


# ---------------------------------------------------------------- bass program


def _build(nsteps=T):
    import os
    dbg = set(x for x in os.environ.get("KDBG", "").split(",") if x)
    no_trans = "notrans" in dbg
    no_bias = "nobias" in dbg
    no_ew = "noew" in dbg
    nc = bacc.Bacc("TRN2", target_bir_lowering=False, debug=False,
                   num_devices=NCORES)

    Wp = nc.dram_tensor("Wp", [KC, 4, 128, 4096], FP16, kind="ExternalInput").ap()
    EWp = nc.dram_tensor("EWp", [2, KC, NOPS, 1024], FP16, kind="ExternalInput").ap()
    WSp = nc.dram_tensor("WSp", [128, 2, 3200], FP16, kind="ExternalInput").ap()
    BS0 = nc.dram_tensor("BS0", [1, 2, NOPS], FP16, kind="ExternalInput").ap()
    BS1 = nc.dram_tensor("BS1", [1, 2, NOPS], FP16, kind="ExternalInput").ap()
    BG = nc.dram_tensor("BG", [128, 64], FP32, kind="ExternalInput").ap()
    BGH = nc.dram_tensor("BGH", [128, 64], FP32, kind="ExternalInput").ap()
    Gt = nc.dram_tensor("Gt", [T, BT, 128, NOPS], FP32, kind="ExternalInput").ap()
    OH0 = nc.dram_tensor("OH0", [128, BL], FP16, kind="ExternalInput").ap()
    REVI = nc.dram_tensor("REVI", [128, 128], FP32, kind="ExternalInput").ap()
    IDENT = nc.dram_tensor("IDENT", [128, 128], FP32, kind="ExternalInput").ap()
    ONES = nc.dram_tensor("ONES", [1, 128], FP16, kind="ExternalInput").ap()
    OUT = nc.dram_tensor("OUT", [BL, T, NOPS], FP32, kind="ExternalOutput").ap()

    with tile.TileContext(nc) as tc, ExitStack() as ctx:
        const = ctx.enter_context(tc.tile_pool(name="const", bufs=1))
        state = ctx.enter_context(tc.tile_pool(name="state", bufs=1))
        wpool = ctx.enter_context(tc.tile_pool(name="wpool", bufs=4))
        ewpool = ctx.enter_context(tc.tile_pool(name="ewpool", bufs=3))
        gpool = ctx.enter_context(tc.tile_pool(name="gpool", bufs=8))
        gates = ctx.enter_context(tc.tile_pool(name="gates", bufs=2))
        scr = ctx.enter_context(tc.tile_pool(name="scr", bufs=1))
        smp = ctx.enter_context(tc.tile_pool(name="smp", bufs=2))
        ohpool = ctx.enter_context(tc.tile_pool(name="ohpool", bufs=2))
        psA = ctx.enter_context(tc.tile_pool(name="psA", bufs=2, space="PSUM"))
        psB = ctx.enter_context(tc.tile_pool(name="psB", bufs=2, space="PSUM"))
        psL = ctx.enter_context(tc.tile_pool(name="psL", bufs=1, space="PSUM"))
        psT = ctx.enter_context(tc.tile_pool(name="psT", bufs=1, space="PSUM"))

        revi = const.tile([128, 128], FP32, name="revi")
        nc.sync.dma_start(revi[:], REVI[:, :])
        ident = const.tile([128, 128], FP32, name="ident")
        nc.sync.dma_start(ident[:], IDENT[:, :])
        ones = const.tile([1, 128], FP16, name="ones")
        nc.sync.dma_start(ones[:], ONES[:, :])
        bg = const.tile([128, 64], FP32, name="bg")
        nc.sync.dma_start(bg[:], BG[:, :])
        bgh = const.tile([128, 64], FP32, name="bgh")
        nc.sync.dma_start(bgh[:], BGH[:, :])
        wsel = const.tile([128, 2, 3200], FP16, name="wsel")
        nc.sync.dma_start(wsel[:], WSp[:, :, :])
        bs0 = const.tile([1, 2, NOPS], FP16, name="bs0")
        nc.sync.dma_start(bs0[:], BS0[:, :, :])
        bs1 = const.tile([1, 2, NOPS], FP16, name="bs1")
        nc.sync.dma_start(bs1[:], BS1[:, :, :])

        cT = state.tile([128, KC, 512], FP32, name="cT")
        h0 = [state.tile([128, KC, 512], FP16, name=f"h0_{i}") for i in range(2)]
        h1 = [state.tile([128, KC, 512], FP16, name=f"h1_{i}") for i in range(2)]

        oh_prev = ohpool.tile([128, 512], FP16, tag="oh", name="oh_init")
        nc.sync.dma_start(oh_prev[:], OH0[:, :])

        for t in range(nsteps):
            par = t % 2
            rpar = (t - 1) % 2

            # ---------------- gates + LSTM elementwise, per H-chunk ----------
            for kh in range(KC):
                ewt = ewpool.tile([NOPS, 1024], FP16, tag="ew", name=f"ew_{t}_{kh}")
                nc.sync.dma_start(ewt[:], EWp[par, kh])

                gt4 = []
                for g in range(4):
                    m = g * 16 + kh
                    wt = wpool.tile([128, 4096], FP16, tag="w", name=f"w_{t}_{m}")
                    nc.sync.dma_start(wt[:], Wp[kh, g])
                    pm = psA.tile([128, 512], FP32, tag="pm", name=f"pm_{t}_{m}")
                    pc = psB.tile([128, 512], FP32, tag="pc", name=f"pc_{t}_{m}")
                    if t > 0:
                        for kc in range(KC):
                            nc.tensor.matmul(pm[:], wt[:, kc * 128:(kc + 1) * 128],
                                             h0[rpar][:, kc, :],
                                             start=(kc == 0), stop=False)
                        for kc in range(KC):
                            nc.tensor.matmul(pc[:], wt[:, kc * 128:(kc + 1) * 128],
                                             h1[rpar][:, kc, :],
                                             start=(kc == 0), stop=False)
                        for kc in range(KC):
                            nc.tensor.matmul(pc[:], wt[:, (16 + kc) * 128:(17 + kc) * 128],
                                             h0[rpar][:, kc, :],
                                             start=False, stop=False)
                    # fused-embedding contribution, ordered last (depends on
                    # the previous step's sampled one-hot)
                    if no_ew and t > 0:
                        # close the accumulation groups without the EW term
                        nc.tensor.matmul(pm[:], wt[:, 0:128], h0[rpar][:, 0, :],
                                         start=False, stop=True)
                        nc.tensor.matmul(pc[:], wt[:, 0:128], h1[rpar][:, 0, :],
                                         start=False, stop=True)
                    else:
                        nc.tensor.matmul(pm[:], ewt[:, g * 128:(g + 1) * 128],
                                         oh_prev[0:NOPS, :], start=(t == 0), stop=True)
                        nc.tensor.matmul(pc[:], ewt[:, (4 + g) * 128:(5 + g) * 128],
                                         oh_prev[0:NOPS, :], start=(t == 0), stop=True)
                    pms = gates.tile([128, 512], FP32, tag="pms", name=f"pms_{t}_{m}")
                    nc.scalar.copy(pms[:], pm[:])
                    gt_ = gates.tile([128, 512], FP32, tag=f"g{g}", name=f"gt_{t}_{m}")
                    nc.vector.scalar_tensor_tensor(gt_[:], pc[:], 1.0 / SC, pms[:],
                                                   OP.mult, OP.add)
                    gt4.append(gt_)

                gi, gf, gg, go = gt4
                mi, mf, mg, mo = kh, 16 + kh, 32 + kh, 48 + kh
                tx_i = scr.tile([128, 512], FP32, tag="tx_i", name=f"txi_{t}_{kh}")
                nc.scalar.activation(tx_i[:], gi[:], AF.Tanh,
                                     bias=bgh[:, mi:mi + 1], scale=0.5)
                si = scr.tile([128, 512], FP32, tag="si", name=f"si_{t}_{kh}")
                nc.vector.tensor_scalar(si[:], tx_i[:], 0.5, 0.5, OP.mult, OP.add)
                tx_f = scr.tile([128, 512], FP32, tag="tx_f", name=f"txf_{t}_{kh}")
                nc.scalar.activation(tx_f[:], gf[:], AF.Tanh,
                                     bias=bgh[:, mf:mf + 1], scale=0.5)
                sf = scr.tile([128, 512], FP32, tag="sf", name=f"sf_{t}_{kh}")
                nc.vector.tensor_scalar(sf[:], tx_f[:], 0.5, 0.5, OP.mult, OP.add)
                tg = scr.tile([128, 512], FP32, tag="tg", name=f"tg_{t}_{kh}")
                nc.scalar.activation(tg[:], gg[:], AF.Tanh,
                                     bias=bg[:, mg:mg + 1], scale=1.0)
                tx_o = scr.tile([128, 512], FP32, tag="tx_o", name=f"txo_{t}_{kh}")
                nc.scalar.activation(tx_o[:], go[:], AF.Tanh,
                                     bias=bgh[:, mo:mo + 1], scale=0.5)
                so = scr.tile([128, 512], FP32, tag="so", name=f"so_{t}_{kh}")
                nc.vector.tensor_scalar(so[:], tx_o[:], 0.5, 0.5, OP.mult, OP.add)

                cs = cT[:, kh, :]
                if t == 0:
                    nc.vector.tensor_mul(cs, si[:], tg[:])
                else:
                    t1 = scr.tile([128, 512], FP32, tag="t1", name=f"t1_{t}_{kh}")
                    nc.vector.tensor_mul(t1[:], sf[:], cs)
                    t2 = scr.tile([128, 512], FP32, tag="t2", name=f"t2_{t}_{kh}")
                    nc.vector.tensor_mul(t2[:], si[:], tg[:])
                    nc.vector.tensor_add(cs, t1[:], t2[:])
                tc_ = scr.tile([128, 512], FP32, tag="tc", name=f"tc_{t}_{kh}")
                nc.scalar.activation(tc_[:], cs, AF.Tanh)
                th = scr.tile([128, 512], FP32, tag="th", name=f"th_{t}_{kh}")
                nc.vector.tensor_mul(th[:], so[:], tc_[:])
                nc.vector.tensor_copy(h0[par][:, kh, :], th[:])
                dh = scr.tile([128, 512], FP32, tag="dh", name=f"dh_{t}_{kh}")
                nc.vector.tensor_sub(dh[:], th[:], h0[par][:, kh, :])
                nc.vector.tensor_scalar_mul(h1[par][:, kh, :], dh[:], SC)

            # ---------------- logits, probs, sampling, per batch tile --------
            oh_cur = ohpool.tile([128, 512], FP16, tag="oh", name=f"oh_{t}")
            for bt in range(BT):
                c0, c1 = bt * 128, (bt + 1) * 128
                pmL = psL.tile([128, NOPS], FP32, tag="Lm", name=f"pmL_{t}_{bt}")
                pcL = psL.tile([128, NOPS], FP32, tag="Lc", name=f"pcL_{t}_{bt}")
                for kc in range(KC):
                    lhs0 = h0[par][:, kc, c0:c1]
                    lhs1 = h1[par][:, kc, c0:c1]
                    rhs0 = wsel[:, par, kc * NOPS:(kc + 1) * NOPS]
                    rhs1 = wsel[:, par, (16 + kc) * NOPS:(17 + kc) * NOPS]
                    nc.tensor.matmul(pmL[:], lhs0, rhs0, start=(kc == 0), stop=False)
                    nc.tensor.matmul(pcL[:], lhs0, rhs1, start=(kc == 0), stop=False)
                    nc.tensor.matmul(pcL[:], lhs1, rhs0, start=False, stop=False)
                if not no_bias:
                    nc.tensor.matmul(pmL[:], ones[:], bs0[:, par, :],
                                     start=False, stop=True)
                    nc.tensor.matmul(pcL[:], ones[:], bs1[:, par, :],
                                     start=False, stop=True)
                else:
                    nc.tensor.matmul(pmL[:], h0[par][:, 0, c0:c1],
                                     wsel[:, par, 0:NOPS], start=False, stop=True)
                    nc.tensor.matmul(pcL[:], h1[par][:, 0, c0:c1],
                                     wsel[:, par, 0:NOPS], start=False, stop=True)

                pmLs = smp.tile([128, NOPS], FP32, tag="pmLs", name=f"pmLs_{t}_{bt}")
                nc.scalar.copy(pmLs[:], pmL[:])
                lg = smp.tile([128, NOPS], FP32, tag="lg", name=f"lg_{t}_{bt}")
                nc.vector.scalar_tensor_tensor(lg[:], pcL[:], 1.0 / SC, pmLs[:],
                                               OP.mult, OP.add)
                # probs = softmax(lg) -> OUT
                rmax = smp.tile([128, 1], FP32, tag="rmax", name=f"rmax_{t}_{bt}")
                nc.vector.reduce_max(rmax[:], lg[:], axis=mybir.AxisListType.X)
                nmax = smp.tile([128, 1], FP32, tag="nmax", name=f"nmax_{t}_{bt}")
                nc.vector.tensor_scalar_mul(nmax[:], rmax[:], -1.0)
                eu = smp.tile([128, NOPS], FP32, tag="eu", name=f"eu_{t}_{bt}")
                nc.scalar.activation(eu[:], lg[:], AF.Exp, bias=nmax[:], scale=1.0)
                ssum = smp.tile([128, 1], FP32, tag="ssum", name=f"ss_{t}_{bt}")
                nc.vector.reduce_sum(ssum[:], eu[:], axis=mybir.AxisListType.X)
                rrec = smp.tile([128, 1], FP32, tag="rrec", name=f"rr_{t}_{bt}")
                nc.vector.reciprocal(rrec[:], ssum[:])
                pr = smp.tile([128, NOPS], FP32, tag="pr", name=f"pr_{t}_{bt}")
                nc.vector.tensor_scalar_mul(pr[:], eu[:], rrec[:])
                nc.sync.dma_start(OUT[c0:c1, t, :], pr[:])

                # tok = argmax(lg + G); one-hot^T for the next step
                if t < nsteps - 1:
                    gti = gpool.tile([128, NOPS], FP32, tag="G", name=f"G_{t}_{bt}")
                    nc.sync.dma_start(gti[:], Gt[t, bt])
                    sc_ = smp.tile([128, NOPS], FP32, tag="sc", name=f"sc_{t}_{bt}")
                    nc.vector.tensor_add(sc_[:], lg[:], gti[:])
                    smax = smp.tile([128, 1], FP32, tag="smax", name=f"sm_{t}_{bt}")
                    nc.vector.reduce_max(smax[:], sc_[:], axis=mybir.AxisListType.X)
                    eq = smp.tile([128, NOPS], FP32, tag="eq", name=f"eq_{t}_{bt}")
                    nc.vector.tensor_scalar(eq[:], sc_[:], smax[:], None, OP.is_equal)
                    iv = smp.tile([128, NOPS], FP32, tag="iv", name=f"iv_{t}_{bt}")
                    nc.vector.tensor_mul(iv[:], eq[:], revi[:, 0:NOPS])
                    m2 = smp.tile([128, 1], FP32, tag="m2", name=f"m2_{t}_{bt}")
                    nc.vector.reduce_max(m2[:], iv[:], axis=mybir.AxisListType.X)
                    ohb = smp.tile([128, 128], FP16, tag="ohb", name=f"ohb_{t}_{bt}")
                    nc.vector.tensor_scalar(ohb[:], revi[:], m2[:], None, OP.is_equal)
                    if no_trans:
                        nc.vector.memset(oh_cur[:, c0:c1], 0.0)
                    else:
                        nc.sync.dma_start_transpose(oh_cur[:, c0:c1], ohb[:])
            oh_prev = oh_cur

    nc.finalize()
    return nc


# ---------------------------------------------------------------- host packing


def _split16(a):
    a = np.asarray(a, np.float32)
    hi = a.astype(np.float16)
    lo = ((a.astype(np.float64) - hi.astype(np.float64)) * SC).astype(np.float16)
    return hi, lo


def _prep(inputs, nsteps=T):
    f32 = np.float32
    W_ih = np.asarray(inputs["W_ih"], f32)
    W_hh = np.asarray(inputs["W_hh"], f32)
    b_ih = np.asarray(inputs["b_ih"], f32)
    b_hh = np.asarray(inputs["b_hh"], f32)
    op_W = np.asarray(inputs["op_W"], f32)
    op_b = np.asarray(inputs["op_b"], f32)
    mag_W = np.asarray(inputs["mag_W"], f32)
    mag_b = np.asarray(inputs["mag_b"], f32)
    op_emb = np.asarray(inputs["op_emb"], f32)
    mag_emb = np.asarray(inputs["mag_emb"], f32)
    x = np.asarray(inputs["x"]).reshape(-1).astype(np.int64)

    # W_hh slabs: Wpack[kh, g, p, (comp*16 + kc)*128 + c]
    W_hhT = np.ascontiguousarray(W_hh.T)                     # [2048, 8192]
    w0, w1 = _split16(W_hhT)
    packs = []
    for wc in (w0, w1):
        wr = wc.reshape(16, 128, 4, 16, 128)                 # [kc, p, g, kh, c]
        packs.append(wr.transpose(3, 2, 1, 0, 4))            # [kh, g, p, kc, c]
    Wp = np.ascontiguousarray(
        np.stack(packs, axis=3).reshape(KC, 4, 128, 4096))   # comp before kc

    # fused embedding tables: parity 0 (even steps) uses mag_emb
    EWs = []
    for emb in (mag_emb, op_emb):
        EW = (emb.astype(np.float64) @ W_ih.T.astype(np.float64)).astype(f32)
        e0, e1 = _split16(EW)                                # [100, 8192]
        per = []
        for ec in (e0, e1):
            er = ec.reshape(NOPS, 4, 16, 128)                # [:, g, kh, c]
            per.append(er.transpose(2, 0, 1, 3))             # [kh, 100, g, c]
        EWs.append(np.stack(per, axis=2)                     # [kh, 100, comp, g, c]
                   .reshape(KC, NOPS, 1024))
    EWp = np.ascontiguousarray(np.stack(EWs, axis=0))        # [2, 16, 100, 1024]

    # logits weights: parity 0 (even steps) uses op_W
    WSs, B0s, B1s = [], [], []
    for sel_W, sel_b in ((op_W, op_b), (mag_W, mag_b)):
        ST = np.ascontiguousarray(sel_W.T)                   # [2048, 100]
        s0, s1 = _split16(ST)
        per = []
        for sc_ in (s0, s1):
            sr = sc_.reshape(16, 128, NOPS)                  # [kc, p, j]
            per.append(sr.transpose(1, 0, 2))                # [p, kc, j]
        WSs.append(np.stack(per, axis=1).reshape(128, 3200)) # [p, comp, kc, j]
        t0, t1 = _split16(sel_b.reshape(1, NOPS))
        B0s.append(t0)
        B1s.append(t1)
    WSp = np.ascontiguousarray(np.stack(WSs, axis=1))        # [128, 2, 3200]
    BS0 = np.ascontiguousarray(np.stack(B0s, axis=1))        # [1, 2, 100]
    BS1 = np.ascontiguousarray(np.stack(B1s, axis=1))

    bgv = (b_ih + b_hh).astype(f32)                          # [8192]
    BG = np.ascontiguousarray(bgv.reshape(64, 128).T)        # [p, m]
    BGH = np.ascontiguousarray(0.5 * BG)

    G = _gumbel_table()                                      # [T, 4096, 100] f32

    rv = np.zeros(128, f32)
    rv[:NOPS] = NOPS - np.arange(NOPS, dtype=f32)
    REVI = np.broadcast_to(rv[None, :], (128, 128)).copy()
    IDENT = np.eye(128, dtype=f32)
    ONES = np.ones((1, 128), np.float16)

    common = dict(Wp=Wp, EWp=EWp, WSp=WSp, BS0=BS0, BS1=BS1, BG=BG, BGH=BGH,
                  REVI=REVI, IDENT=IDENT, ONES=ONES)
    in_maps = []
    for c in range(NCORES):
        tok = x[c * BL:(c + 1) * BL]
        oh0 = (tok[None, :] == np.arange(128)[:, None]).astype(np.float16)
        Gc = np.ascontiguousarray(
            G[:, c * BL:(c + 1) * BL, :].reshape(T, BT, 128, NOPS))
        in_maps.append(dict(common, OH0=oh0, Gt=Gc))
    return in_maps


_G_CACHE = None


def _gumbel_table():
    global _G_CACHE
    if _G_CACHE is None:
        import jax
        cpu = jax.devices("cpu")[0]
        with jax.default_device(cpu):
            keys = jax.random.split(jax.random.PRNGKey(42), T)
            g = [np.asarray(jax.random.gumbel(k, (B, NOPS), dtype=np.float32))
                 for k in keys]
        _G_CACHE = np.stack(g)
    return _G_CACHE


# ---------------------------------------------------------------- execution


_RUNNER_CACHE = {}


def _get_runner(nsteps=T):
    """Build + jit once; returns run(in_maps, timing_iters) -> (results, times)."""
    if nsteps in _RUNNER_CACHE:
        return _RUNNER_CACHE[nsteps]

    import jax
    from jax.sharding import Mesh, PartitionSpec, NamedSharding
    from jax.experimental.shard_map import shard_map
    from concourse import bass2jax

    nc = _build(nsteps)
    bass2jax.install_neuronx_cc_hook()

    partition_name = (nc.partition_id_tensor.name
                      if nc.partition_id_tensor else None)
    in_names, out_names, out_avals, zero_shapes = [], [], [], []
    for alloc in nc.m.functions[0].allocations:
        if not isinstance(alloc, mybir.MemoryLocationSet):
            continue
        name = alloc.memorylocations[0].name
        if alloc.kind == "ExternalInput":
            if name != partition_name:
                in_names.append(name)
        elif alloc.kind == "ExternalOutput":
            out_names.append(name)
            shape = tuple(alloc.tensor_shape)
            dtype = mybir.dt.np(alloc.dtype)
            out_avals.append(jax.core.ShapedArray(shape, dtype))
            zero_shapes.append((shape, dtype))
    n_params = len(in_names)
    all_names = in_names + out_names
    if partition_name is not None:
        all_names = all_names + [partition_name]
    donate = tuple(range(n_params, n_params + len(out_names)))

    def _body(*args):
        operands = list(args)
        if partition_name is not None:
            operands.append(bass2jax.partition_id_tensor())
        outs = bass2jax._bass_exec_p.bind(
            *operands,
            out_avals=tuple(out_avals),
            in_names=tuple(all_names),
            out_names=tuple(out_names),
            lowering_input_output_aliases=(),
            sim_require_finite=True,
            sim_require_nnan=True,
            nc=nc,
        )
        return tuple(outs)

    devices = jax.devices()[:NCORES]
    mesh = Mesh(np.asarray(devices), ("core",))
    nio = n_params + len(out_names)
    sharded = jax.jit(
        shard_map(_body, mesh=mesh,
                  in_specs=(PartitionSpec("core"),) * nio,
                  out_specs=(PartitionSpec("core"),) * len(out_names),
                  check_rep=False),
        donate_argnums=donate, keep_unused=True)
    shard = NamedSharding(mesh, PartitionSpec("core"))

    def run(in_maps, timing_iters=0):
        concat_in = [
            np.concatenate([np.asarray(in_maps[c][n]) for c in range(NCORES)], axis=0)
            for n in in_names
        ]
        dev_in = [jax.device_put(a, shard) for a in concat_in]
        jax.block_until_ready(dev_in)

        def zeros():
            return [jax.device_put(np.zeros((NCORES * s[0], *s[1:]), d), shard)
                    for (s, d) in zero_shapes]

        z = zeros()
        jax.block_until_ready(z)
        out = sharded(*dev_in, *z)
        jax.block_until_ready(out)
        times = []
        for _ in range(timing_iters):
            z = zeros()
            jax.block_until_ready(z)
            t0 = time.perf_counter()
            o2 = sharded(*dev_in, *z)
            jax.block_until_ready(o2)
            times.append(time.perf_counter() - t0)
        res = np.asarray(out[0])
        per_core = res.reshape(NCORES, BL, T, NOPS)
        return per_core, times

    _RUNNER_CACHE[nsteps] = run
    return run


def kernel(**inputs) -> np.ndarray:
    in_maps = _prep(inputs)
    run = _get_runner(T)
    per_core, _ = run(in_maps, timing_iters=0)
    return np.ascontiguousarray(per_core.reshape(B, T, NOPS))


if __name__ == "__main__":
    import reference
    ins = reference.setup_inputs()
    out = kernel(**ins)
    print(out.shape, out.dtype)
